# revision 25
# baseline (speedup 1.0000x reference)
"""Multi-head attention (B=4, N=2048, D=1024, H=16) on 8 Trainium2 NeuronCores.

Sharding: tensor-parallel over heads — 2 heads per core. Each core computes
QKV^T for its heads from the (host-pre-transposed) full X^T, runs attention,
and produces a partial projection output (its 128 rows of w_proj). The host
sums the 8 partial outputs.

Layout trick: everything stays "transposed" (feature dim on partitions):
  QKV^T [128=(h0 dims|h1 dims), tok] = W_chunk.T @ XT_chunk     (fp32r, N=512)
  S^T   [keys, q]  = (K^T_h chunk).T @ Q^T_h                    (64-contraction,
                      two heads packed in array row halves via tile_position)
  P^T   = exp(S^T)          (ScalarE, no max subtraction: logits ~ N(0,1))
  O^T   [65, q]    = V_nat_aug.T @ P^T   (V natural layout via PE transpose,
                      65th row = ones column -> softmax denominator for free)
  Y     [tok, 512] = Ofin_chunk.T @ Wp   (128-contraction per core)
"""

import sys
from contextlib import ExitStack

import numpy as np

for _p in ("/opt/trn_rl_repo", "/opt/pypackages"):
    if _p not in sys.path:
        sys.path.insert(0, _p)

B, N, D = 4, 2048, 1024
H, DH = 16, 64
NCORES = 8
HPC = H // NCORES  # heads per core = 2
P = 128
QB = 512  # moving free dim (query block / token block)

_cache = {}


def _build(nbatch, ntok):
    """Build + compile the per-core bass program. Same program on all cores;
    per-core weights arrive as data."""
    import concourse.bacc as bacc
    import concourse.bass as bass
    import concourse.mybir as mybir
    import concourse.tile as tile
    from concourse.masks import make_identity

    f32 = mybir.dt.float32
    f32r = mybir.dt.float32r
    Exp = mybir.ActivationFunctionType.Exp

    DCH = D // P          # 8 contraction chunks for the projections
    nqb = ntok // QB      # query blocks per batch
    nkb = ntok // P       # key blocks per batch
    ntb = ntok // P       # stage-C token blocks per batch
    neb = D // QB         # output col blocks (2)

    nc = bacc.Bacc("TRN2", target_bir_lowering=False, debug=False)

    xt_d = nc.dram_tensor("xt", [D, nbatch * ntok], f32r, kind="ExternalInput")
    wqkv_d = nc.dram_tensor("wqkv", [D, 3 * P], f32r, kind="ExternalInput")
    bq_d = nc.dram_tensor("bq", [P, 3], f32, kind="ExternalInput")
    wp_d = nc.dram_tensor("wp", [P, D], f32r, kind="ExternalInput")
    y_d = nc.dram_tensor("y", [nbatch * ntok, D], f32, kind="ExternalOutput")

    with tile.TileContext(nc) as tc, ExitStack() as ctx:
        const = ctx.enter_context(tc.tile_pool(name="const", bufs=1))
        xt_pool = ctx.enter_context(tc.tile_pool(name="xt", bufs=16))
        qkvt_pool = ctx.enter_context(tc.tile_pool(name="qkvt", bufs=2))
        vn_pool = ctx.enter_context(tc.tile_pool(name="vn", bufs=2))
        pt_pool = ctx.enter_context(tc.tile_pool(name="pt", bufs=4))
        ofin_pool = ctx.enter_context(tc.tile_pool(name="ofin", bufs=2))
        sm_pool = ctx.enter_context(tc.tile_pool(name="sm", bufs=4))
        yo_pool = ctx.enter_context(tc.tile_pool(name="yo", bufs=6))
        ps512 = ctx.enter_context(tc.tile_pool(name="ps512", bufs=2, space="PSUM"))
        pst = ctx.enter_context(tc.tile_pool(name="pst", bufs=2, space="PSUM"))
        pso = ctx.enter_context(tc.tile_pool(name="pso", bufs=2, space="PSUM"))

        # ---- constants ----
        w_sb = const.tile([P, DCH * 3 * P], f32r, tag="w")
        for d in range(DCH):
            nc.sync.dma_start(
                w_sb[:, d * 384 : (d + 1) * 384], wqkv_d[d * P : (d + 1) * P, :]
            )
        wp_sb = const.tile([P, D], f32r, tag="wp")
        nc.sync.dma_start(wp_sb[:], wp_d[:])
        bq_sb = const.tile([P, 3], f32, tag="bq")
        nc.sync.dma_start(bq_sb[:], bq_d[:])
        ident = const.tile([P, P], f32, tag="ident")
        make_identity(nc, ident)
        # ones column for the V-augmentation (softmax denominator row)
        vcol_f = const.tile([P, 1], f32, tag="vcol_f")
        nc.vector.memset(vcol_f[:], 1.0)
        # ones row — broadcasts a [1, QB] reciprocal row across 64 partitions
        ones_f = const.tile([1, DH], f32, tag="ones_f")
        nc.vector.memset(ones_f[:], 1.0)
        ones_sb = const.tile([1, DH], f32r, tag="ones")
        nc.vector.tensor_copy(ones_sb[:], ones_f[:])

        nA = ntok // QB  # token groups (stage A) == query blocks (stage B)

        def stage_a_tok(bt, tb):
            """QKV^T for token block tb of batch bt."""
            t0 = bt * ntok
            qt, kt, vt = bt_tiles[bt]
            dest = {0: qt, 1: kt, 2: vt}
            xts = []
            for d in range(DCH):
                t = xt_pool.tile([P, QB], f32r, tag="xt", name=f"xt{d}")
                nc.sync.dma_start(
                    t[:], xt_d[d * P : (d + 1) * P, t0 + tb * QB : t0 + (tb + 1) * QB]
                )
                xts.append(t)
            for cb in range(3):
                ps = ps512.tile([P, QB], f32, tag="ps", name="psA")
                for d in range(DCH):
                    nc.tensor.matmul(
                        ps[:],
                        w_sb[:, d * 384 + cb * P : d * 384 + (cb + 1) * P],
                        xts[d][:],
                        start=(d == 0),
                        stop=(d == DCH - 1),
                    )
                # psum -> sbuf, adding the (per-output-column) qkv bias
                nc.vector.tensor_scalar_add(
                    dest[cb][:, tb * QB : (tb + 1) * QB], ps[:], bq_sb[:, cb : cb + 1]
                )

        def build_vn_group(bt, g, ngroups):
            """PE-transpose V^T -> V natural (+ones col), for key blocks of group g."""
            _, _, vt = bt_tiles[bt]
            vn = vn_tiles[bt]
            for kb in range(g * nkb // ngroups, (g + 1) * nkb // ngroups):
                trp = ps512.tile([P, P], f32, tag="ps", name="trp")
                nc.tensor.transpose(trp[:], vt[:, kb * P : (kb + 1) * P], ident[:])
                for h in range(HPC):
                    nc.vector.tensor_copy(
                        vn[:, (h * nkb + kb) * 65 : (h * nkb + kb) * 65 + 64],
                        trp[:, h * 64 : (h + 1) * 64],
                    )
                for h in range(HPC):
                    idx = (h * nkb + kb + 1) * 65 - 1
                    nc.vector.tensor_copy(vn[:, idx : idx + 1], vcol_f[:])

        def stage_b_qblock(bt, qb):
            """Attention for query block qb of batch bt, both heads."""
            qt, kt, _ = bt_tiles[bt]
            vn = vn_tiles[bt]
            ofin = ofin_tiles[bt]
            q0, q1 = qb * QB, (qb + 1) * QB
            oacc = [
                pso.tile([65, QB], f32, tag="oacc", name=f"oacc{_h}")
                for _h in range(HPC)
            ]
            rc = [
                sm_pool.tile([1, QB], f32r, tag=f"rc{_h}", name=f"rc{_h}")
                for _h in range(HPC)
            ]
            for kb in range(nkb):
                st = pst.tile([P, HPC * QB], f32, tag="st", name="st")
                for h in range(HPC):
                    nc.tensor.matmul(
                        st[:, h * QB : (h + 1) * QB],
                        kt[h * 64 : (h + 1) * 64, kb * P : (kb + 1) * P],
                        qt[h * 64 : (h + 1) * 64, q0:q1],
                        start=True,
                        stop=True,
                        tile_position=(h * 64, 0),
                    )
                ptile = pt_pool.tile([P, HPC * QB], f32r, tag="pt", name="pt")
                nc.scalar.activation(ptile[:], st[:], Exp)
                for h in range(HPC):
                    nc.tensor.matmul(
                        oacc[h][:],
                        vn[:, (h * nkb + kb) * 65 : (h * nkb + kb + 1) * 65],
                        ptile[:, h * QB : (h + 1) * QB],
                        start=(kb == 0),
                        stop=(kb == nkb - 1),
                        skip_group_check=True,
                    )
            bc_sb = sm_pool.tile([P, QB], f32, tag="bc", name="bc_sb")
            for h in range(HPC):
                with nc.allow_low_precision(reason="f32r feed to bcast matmul"):
                    nc.vector.reciprocal(rc[h][:], oacc[h][64:65, :])
                bc_ps = ps512.tile([DH, QB], f32, tag="ps", name=f"bc_ps{h}")
                nc.tensor.matmul(bc_ps[:], ones_sb[:], rc[h][:], start=True, stop=True)
                nc.vector.tensor_copy(bc_sb[h * 64 : (h + 1) * 64, :], bc_ps[:])
            for h in range(HPC):
                nc.vector.tensor_mul(
                    ofin[h * 64 : (h + 1) * 64, q0:q1],
                    oacc[h][0:64, :],
                    bc_sb[h * 64 : (h + 1) * 64, :],
                )

        def stage_c_group(bt, g, ngroups):
            """Partial projection for token blocks of group g (needs ofin of the
            matching qblock only)."""
            t0 = bt * ntok
            ofin = ofin_tiles[bt]
            for tb in range(g * ntb // ngroups, (g + 1) * ntb // ngroups):
                for eb in range(neb):
                    yp = ps512.tile([P, QB], f32, tag="ps", name="yp")
                    nc.tensor.matmul(
                        yp[:],
                        ofin[:, tb * P : (tb + 1) * P],
                        wp_sb[:, eb * QB : (eb + 1) * QB],
                        start=True,
                        stop=True,
                    )
                    yo = yo_pool.tile([P, QB], f32, tag="yo", name="yo")
                    nc.vector.tensor_copy(yo[:], yp[:])
                    nc.sync.dma_start(
                        y_d[t0 + tb * P : t0 + (tb + 1) * P, eb * QB : (eb + 1) * QB],
                        yo[:],
                    )

        # ---- software-pipelined emission: stage A of batch b+1 interleaves
        # with stage B/C of batch b so every engine's (in-order) stream
        # alternates phases and DMA/PE/ACT overlap across batches ----
        bt_tiles = {}
        vn_tiles = {}
        ofin_tiles = {}
        for step in range(nbatch + 1):
            if step < nbatch:
                qt = qkvt_pool.tile([P, ntok], f32r, tag="qt", name="qt")
                kt = qkvt_pool.tile([P, ntok], f32r, tag="kt", name="kt")
                vt = qkvt_pool.tile([P, ntok], f32, tag="vt", name="vt")
                bt_tiles[step] = (qt, kt, vt)
                vn_tiles[step] = vn_pool.tile(
                    [P, HPC * nkb * 65], f32r, tag="vn", name="vn"
                )
                ofin_tiles[step] = ofin_pool.tile(
                    [P, ntok], f32r, tag="ofin", name="ofin"
                )
            for i in range(nA):
                if step < nbatch:
                    stage_a_tok(step, i)
                    build_vn_group(step, i, nA)
                if step >= 1:
                    stage_b_qblock(step - 1, i)
                    stage_c_group(step - 1, i, nA)
            # free batch tiles we no longer need
            if step >= 1:
                for dct in (bt_tiles, vn_tiles, ofin_tiles):
                    dct.pop(step - 1, None)

    nc.compile()
    return nc


def get_compiled(nbatch=B, ntok=N):
    key = (nbatch, ntok)
    if key not in _cache:
        _cache[key] = _build(nbatch, ntok)
    return _cache[key]


def make_core_inputs(x, w_qkv, b_qkv, w_proj):
    """Host-side sharding: returns (in_maps list for 8 cores)."""
    B_, N_, D_ = x.shape
    xt = np.ascontiguousarray(x.reshape(B_ * N_, D_).T).astype(np.float32)
    in_maps = []
    for c in range(NCORES):
        heads = [HPC * c + i for i in range(HPC)]

        def wcols(s, scale=1.0):
            return np.concatenate(
                [w_qkv[:, s * D + h * DH : s * D + (h + 1) * DH] for h in heads], axis=1
            ) * scale

        def bcol(s, scale=1.0):
            return np.concatenate(
                [b_qkv[s * D + h * DH : s * D + (h + 1) * DH] for h in heads]
            ) * scale

        scale = float(DH) ** -0.5
        wqkv_c = np.ascontiguousarray(
            np.concatenate([wcols(0, scale), wcols(1), wcols(2)], axis=1)
        ).astype(np.float32)
        bq_c = np.stack([bcol(0, scale), bcol(1), bcol(2)], axis=1).astype(np.float32)
        bq_c = np.ascontiguousarray(bq_c)
        wp_c = np.ascontiguousarray(
            np.concatenate([w_proj[h * DH : (h + 1) * DH, :] for h in heads], axis=0)
        ).astype(np.float32)
        in_maps.append({"xt": xt, "wqkv": wqkv_c, "bq": bq_c, "wp": wp_c})
    return in_maps


def kernel(x, w_qkv, b_qkv, w_proj, b_proj):
    x = np.asarray(x, dtype=np.float32)
    w_qkv = np.asarray(w_qkv, dtype=np.float32)
    b_qkv = np.asarray(b_qkv, dtype=np.float32)
    w_proj = np.asarray(w_proj, dtype=np.float32)
    b_proj = np.asarray(b_proj, dtype=np.float32)
    B_, N_, D_ = x.shape

    from concourse.bass_utils import run_bass_kernel_spmd

    nc = get_compiled(B_, N_)
    in_maps = make_core_inputs(x, w_qkv, b_qkv, w_proj)
    res = run_bass_kernel_spmd(nc, in_maps, core_ids=list(range(NCORES)))
    y = res.results[0]["y"].astype(np.float64)
    for r in res.results[1:]:
        y = y + r["y"].astype(np.float64)
    y = y + b_proj[None, :].astype(np.float64)
    return y.reshape(B_, N_, D_).astype(np.float32)


# revision 26
# speedup vs baseline: 10.0991x; 10.0991x over previous
"""Multi-head attention (B=4, N=2048, D=1024, H=16) on 8 Trainium2 NeuronCores.

Sharding: tensor-parallel over heads — 2 heads per core. Each core computes
QKV^T for its heads from the (host-pre-transposed) full X^T, runs attention,
and produces a partial projection output (its 128 rows of w_proj). The host
sums the 8 partial outputs.

Layout trick: everything stays "transposed" (feature dim on partitions):
  QKV^T [128=(h0 dims|h1 dims), tok] = W_chunk.T @ XT_chunk     (fp32r, N=512)
  S^T   [keys, q]  = (K^T_h chunk).T @ Q^T_h                    (64-contraction,
                      two heads packed in array row halves via tile_position)
  P^T   = exp(S^T)          (ScalarE, no max subtraction: logits ~ N(0,1))
  O^T   [65, q]    = V_nat_aug.T @ P^T   (V natural layout via PE transpose,
                      65th row = ones column -> softmax denominator for free)
  Y     [tok, 512] = Ofin_chunk.T @ Wp   (128-contraction per core)
"""

import sys
from contextlib import ExitStack

import numpy as np

for _p in ("/opt/trn_rl_repo", "/opt/pypackages"):
    if _p not in sys.path:
        sys.path.insert(0, _p)

B, N, D = 4, 2048, 1024
H, DH = 16, 64
NCORES = 8
HPC = H // NCORES  # heads per core = 2
P = 128
QB = 512  # moving free dim (query block / token block)

_cache = {}


def _build(nbatch, ntok):
    """Build + compile the per-core bass program. Same program on all cores;
    per-core weights arrive as data."""
    import concourse.bacc as bacc
    import concourse.mybir as mybir
    import concourse.tile as tile
    from concourse.masks import make_identity

    f32 = mybir.dt.float32
    f32r = mybir.dt.float32r
    Exp = mybir.ActivationFunctionType.Exp

    DCH = D // P          # 8 contraction chunks for the projections
    nqb = ntok // QB      # query blocks per batch
    nkb = ntok // P       # key blocks per batch
    ntb = ntok // P       # stage-C token blocks per batch
    neb = D // QB         # output col blocks (2)

    nc = bacc.Bacc("TRN2", target_bir_lowering=False, debug=False)

    xt_d = nc.dram_tensor("xt", [D, nbatch * ntok], f32r, kind="ExternalInput")
    wqkv_d = nc.dram_tensor("wqkv", [D, 3 * P], f32r, kind="ExternalInput")
    bq_d = nc.dram_tensor("bq", [P, 3], f32, kind="ExternalInput")
    wp_d = nc.dram_tensor("wp", [P, D], f32r, kind="ExternalInput")
    y_d = nc.dram_tensor("y", [nbatch * ntok, D], f32, kind="ExternalOutput")

    with tile.TileContext(nc) as tc, ExitStack() as ctx:
        const = ctx.enter_context(tc.tile_pool(name="const", bufs=1))
        xt_pool = ctx.enter_context(tc.tile_pool(name="xt", bufs=16))
        qkvt_pool = ctx.enter_context(tc.tile_pool(name="qkvt", bufs=2))
        vn_pool = ctx.enter_context(tc.tile_pool(name="vn", bufs=2))
        pt_pool = ctx.enter_context(tc.tile_pool(name="pt", bufs=4))
        ofin_pool = ctx.enter_context(tc.tile_pool(name="ofin", bufs=2))
        sm_pool = ctx.enter_context(tc.tile_pool(name="sm", bufs=4))
        yo_pool = ctx.enter_context(tc.tile_pool(name="yo", bufs=6))
        ps512 = ctx.enter_context(tc.tile_pool(name="ps512", bufs=2, space="PSUM"))
        pst = ctx.enter_context(tc.tile_pool(name="pst", bufs=2, space="PSUM"))
        pso = ctx.enter_context(tc.tile_pool(name="pso", bufs=2, space="PSUM"))

        # ---- constants ----
        w_sb = const.tile([P, DCH * 3 * P], f32r, tag="w")
        for d in range(DCH):
            nc.sync.dma_start(
                w_sb[:, d * 384 : (d + 1) * 384], wqkv_d[d * P : (d + 1) * P, :]
            )
        wp_sb = const.tile([P, D], f32r, tag="wp")
        nc.sync.dma_start(wp_sb[:], wp_d[:])
        bq_sb = const.tile([P, 3], f32, tag="bq")
        nc.sync.dma_start(bq_sb[:], bq_d[:])
        ident = const.tile([P, P], f32, tag="ident")
        make_identity(nc, ident)
        # ones column for the V-augmentation (softmax denominator row)
        vcol_f = const.tile([P, 1], f32, tag="vcol_f")
        nc.vector.memset(vcol_f[:], 1.0)
        # ones row — broadcasts a [1, QB] reciprocal row across 64 partitions
        ones_f = const.tile([1, DH], f32, tag="ones_f")
        nc.vector.memset(ones_f[:], 1.0)
        ones_sb = const.tile([1, DH], f32r, tag="ones")
        nc.vector.tensor_copy(ones_sb[:], ones_f[:])

        nA = ntok // QB  # token groups (stage A) == query blocks (stage B)

        def stage_a_tok(bt, tb):
            """QKV^T for token block tb of batch bt."""
            t0 = bt * ntok
            qt, kt, vt = bt_tiles[bt]
            dest = {0: qt, 1: kt, 2: vt}
            xts = []
            for d in range(DCH):
                t = xt_pool.tile([P, QB], f32r, tag="xt", name=f"xt{d}")
                nc.sync.dma_start(
                    t[:], xt_d[d * P : (d + 1) * P, t0 + tb * QB : t0 + (tb + 1) * QB]
                )
                xts.append(t)
            for cb in range(3):
                ps = ps512.tile([P, QB], f32, tag="ps", name="psA")
                for d in range(DCH):
                    nc.tensor.matmul(
                        ps[:],
                        w_sb[:, d * 384 + cb * P : d * 384 + (cb + 1) * P],
                        xts[d][:],
                        start=(d == 0),
                        stop=(d == DCH - 1),
                    )
                # psum -> sbuf, adding the (per-output-column) qkv bias
                nc.vector.tensor_scalar_add(
                    dest[cb][:, tb * QB : (tb + 1) * QB], ps[:], bq_sb[:, cb : cb + 1]
                )

        def build_vn_group(bt, g, ngroups):
            """PE-transpose V^T -> V natural (+ones col), for key blocks of group g."""
            _, _, vt = bt_tiles[bt]
            vn = vn_tiles[bt]
            for kb in range(g * nkb // ngroups, (g + 1) * nkb // ngroups):
                trp = ps512.tile([P, P], f32, tag="ps", name="trp")
                nc.tensor.transpose(trp[:], vt[:, kb * P : (kb + 1) * P], ident[:])
                for h in range(HPC):
                    nc.vector.tensor_copy(
                        vn[:, (h * nkb + kb) * 65 : (h * nkb + kb) * 65 + 64],
                        trp[:, h * 64 : (h + 1) * 64],
                    )
                for h in range(HPC):
                    idx = (h * nkb + kb + 1) * 65 - 1
                    nc.vector.tensor_copy(vn[:, idx : idx + 1], vcol_f[:])

        def stage_b_qblock(bt, qb):
            """Attention for query block qb of batch bt, both heads."""
            qt, kt, _ = bt_tiles[bt]
            vn = vn_tiles[bt]
            ofin = ofin_tiles[bt]
            q0, q1 = qb * QB, (qb + 1) * QB
            oacc = [
                pso.tile([65, QB], f32, tag="oacc", name=f"oacc{_h}")
                for _h in range(HPC)
            ]
            rc = [
                sm_pool.tile([1, QB], f32r, tag=f"rc{_h}", name=f"rc{_h}")
                for _h in range(HPC)
            ]
            for kb in range(nkb):
                st = pst.tile([P, HPC * QB], f32, tag="st", name="st")
                for h in range(HPC):
                    nc.tensor.matmul(
                        st[:, h * QB : (h + 1) * QB],
                        kt[h * 64 : (h + 1) * 64, kb * P : (kb + 1) * P],
                        qt[h * 64 : (h + 1) * 64, q0:q1],
                        start=True,
                        stop=True,
                        tile_position=(h * 64, 0),
                    )
                ptile = pt_pool.tile([P, HPC * QB], f32r, tag="pt", name="pt")
                nc.scalar.activation(ptile[:], st[:], Exp)
                for h in range(HPC):
                    nc.tensor.matmul(
                        oacc[h][:],
                        vn[:, (h * nkb + kb) * 65 : (h * nkb + kb + 1) * 65],
                        ptile[:, h * QB : (h + 1) * QB],
                        start=(kb == 0),
                        stop=(kb == nkb - 1),
                        skip_group_check=True,
                    )
            bc_sb = sm_pool.tile([P, QB], f32, tag="bc", name="bc_sb")
            for h in range(HPC):
                with nc.allow_low_precision(reason="f32r feed to bcast matmul"):
                    nc.vector.reciprocal(rc[h][:], oacc[h][64:65, :])
                bc_ps = ps512.tile([DH, QB], f32, tag="ps", name=f"bc_ps{h}")
                nc.tensor.matmul(bc_ps[:], ones_sb[:], rc[h][:], start=True, stop=True)
                nc.vector.tensor_copy(bc_sb[h * 64 : (h + 1) * 64, :], bc_ps[:])
            for h in range(HPC):
                nc.vector.tensor_mul(
                    ofin[h * 64 : (h + 1) * 64, q0:q1],
                    oacc[h][0:64, :],
                    bc_sb[h * 64 : (h + 1) * 64, :],
                )

        def stage_c_group(bt, g, ngroups):
            """Partial projection for token blocks of group g (needs ofin of the
            matching qblock only)."""
            t0 = bt * ntok
            ofin = ofin_tiles[bt]
            for tb in range(g * ntb // ngroups, (g + 1) * ntb // ngroups):
                for eb in range(neb):
                    yp = ps512.tile([P, QB], f32, tag="ps", name="yp")
                    nc.tensor.matmul(
                        yp[:],
                        ofin[:, tb * P : (tb + 1) * P],
                        wp_sb[:, eb * QB : (eb + 1) * QB],
                        start=True,
                        stop=True,
                    )
                    yo = yo_pool.tile([P, QB], f32, tag="yo", name="yo")
                    nc.vector.tensor_copy(yo[:], yp[:])
                    nc.sync.dma_start(
                        y_d[t0 + tb * P : t0 + (tb + 1) * P, eb * QB : (eb + 1) * QB],
                        yo[:],
                    )

        # ---- software-pipelined emission: stage A of batch b+1 interleaves
        # with stage B/C of batch b so every engine's (in-order) stream
        # alternates phases and DMA/PE/ACT overlap across batches ----
        bt_tiles = {}
        vn_tiles = {}
        ofin_tiles = {}
        for step in range(nbatch + 1):
            if step < nbatch:
                qt = qkvt_pool.tile([P, ntok], f32r, tag="qt", name="qt")
                kt = qkvt_pool.tile([P, ntok], f32r, tag="kt", name="kt")
                vt = qkvt_pool.tile([P, ntok], f32, tag="vt", name="vt")
                bt_tiles[step] = (qt, kt, vt)
                vn_tiles[step] = vn_pool.tile(
                    [P, HPC * nkb * 65], f32r, tag="vn", name="vn"
                )
                ofin_tiles[step] = ofin_pool.tile(
                    [P, ntok], f32r, tag="ofin", name="ofin"
                )
            for i in range(nA):
                if step < nbatch:
                    stage_a_tok(step, i)
                    build_vn_group(step, i, nA)
                if step >= 1:
                    stage_b_qblock(step - 1, i)
                    stage_c_group(step - 1, i, nA)
            # free batch tiles we no longer need
            if step >= 1:
                for dct in (bt_tiles, vn_tiles, ofin_tiles):
                    dct.pop(step - 1, None)

    nc.compile()
    return nc


def get_compiled(nbatch=B, ntok=N):
    key = (nbatch, ntok)
    if key not in _cache:
        _cache[key] = _build(nbatch, ntok)
    return _cache[key]


def make_core_inputs(x, w_qkv, b_qkv, w_proj):
    """Host-side sharding: returns (in_maps list for 8 cores)."""
    B_, N_, D_ = x.shape
    xt = np.ascontiguousarray(x.reshape(B_ * N_, D_).T).astype(np.float32)
    in_maps = []
    for c in range(NCORES):
        heads = [HPC * c + i for i in range(HPC)]

        def wcols(s, scale=1.0):
            return np.concatenate(
                [w_qkv[:, s * D + h * DH : s * D + (h + 1) * DH] for h in heads], axis=1
            ) * scale

        def bcol(s, scale=1.0):
            return np.concatenate(
                [b_qkv[s * D + h * DH : s * D + (h + 1) * DH] for h in heads]
            ) * scale

        scale = float(DH) ** -0.5
        wqkv_c = np.ascontiguousarray(
            np.concatenate([wcols(0, scale), wcols(1), wcols(2)], axis=1)
        ).astype(np.float32)
        bq_c = np.stack([bcol(0, scale), bcol(1), bcol(2)], axis=1).astype(np.float32)
        bq_c = np.ascontiguousarray(bq_c)
        wp_c = np.ascontiguousarray(
            np.concatenate([w_proj[h * DH : (h + 1) * DH, :] for h in heads], axis=0)
        ).astype(np.float32)
        in_maps.append({"xt": xt, "wqkv": wqkv_c, "bq": bq_c, "wp": wp_c})
    return in_maps


def kernel(x, w_qkv, b_qkv, w_proj, b_proj):
    x = np.asarray(x, dtype=np.float32)
    w_qkv = np.asarray(w_qkv, dtype=np.float32)
    b_qkv = np.asarray(b_qkv, dtype=np.float32)
    w_proj = np.asarray(w_proj, dtype=np.float32)
    b_proj = np.asarray(b_proj, dtype=np.float32)
    B_, N_, D_ = x.shape

    from concourse.bass_utils import run_bass_kernel_spmd

    nc = get_compiled(B_, N_)
    in_maps = make_core_inputs(x, w_qkv, b_qkv, w_proj)
    res = run_bass_kernel_spmd(nc, in_maps, core_ids=list(range(NCORES)))
    y = res.results[0]["y"].astype(np.float64)
    for r in res.results[1:]:
        y = y + r["y"].astype(np.float64)
    y = y + b_proj[None, :].astype(np.float64)
    return y.reshape(B_, N_, D_).astype(np.float32)


# revision 30
# speedup vs baseline: 10.2189x; 1.0119x over previous
"""Multi-head attention (B=4, N=2048, D=1024, H=16) on 8 Trainium2 NeuronCores.

Sharding: tensor-parallel over heads — 2 heads per core. Each core computes
QKV^T for its heads from the (host-pre-transposed) full X^T, runs attention,
and produces a partial projection output (its 128 rows of w_proj). The host
sums the 8 partial outputs.

Layout trick: everything stays "transposed" (feature dim on partitions):
  QKV^T [128=(h0 dims|h1 dims), tok] = W_chunk.T @ XT_chunk     (fp32r, N=512)
  S^T   [keys, q]  = (K^T_h chunk).T @ Q^T_h                    (64-contraction,
                      two heads packed in array row halves via tile_position)
  P^T   = exp(S^T)          (ScalarE, no max subtraction: logits ~ N(0,1))
  O^T   [65, q]    = V_nat_aug.T @ P^T   (V natural layout via PE transpose,
                      65th row = ones column -> softmax denominator for free)
  Y     [tok, 512] = Ofin_chunk.T @ Wp   (128-contraction per core)
"""

import sys
from contextlib import ExitStack

import numpy as np

for _p in ("/opt/trn_rl_repo", "/opt/pypackages"):
    if _p not in sys.path:
        sys.path.insert(0, _p)

B, N, D = 4, 2048, 1024
H, DH = 16, 64
NCORES = 8
HPC = H // NCORES  # heads per core = 2
P = 128
QB = 512  # moving free dim (query block / token block)

_cache = {}


def _build(nbatch, ntok):
    """Build + compile the per-core bass program. Same program on all cores;
    per-core weights arrive as data."""
    import concourse.bacc as bacc
    import concourse.mybir as mybir
    import concourse.tile as tile
    from concourse.masks import make_identity

    f32 = mybir.dt.float32
    f32r = mybir.dt.float32r
    Exp = mybir.ActivationFunctionType.Exp

    DCH = D // P          # 8 contraction chunks for the projections
    nqb = ntok // QB      # query blocks per batch
    nkb = ntok // P       # key blocks per batch
    ntb = ntok // P       # stage-C token blocks per batch
    neb = D // QB         # output col blocks (2)

    nc = bacc.Bacc("TRN2", target_bir_lowering=False, debug=False)

    xt_d = nc.dram_tensor("xt", [D, nbatch * ntok], f32r, kind="ExternalInput")
    wqkv_d = nc.dram_tensor("wqkv", [D, 3 * P], f32r, kind="ExternalInput")
    bq_d = nc.dram_tensor("bq", [P, 3], f32, kind="ExternalInput")
    wp_d = nc.dram_tensor("wp", [P, D], f32r, kind="ExternalInput")
    y_d = nc.dram_tensor("y", [nbatch * ntok, D], f32, kind="ExternalOutput")

    with tile.TileContext(nc) as tc, ExitStack() as ctx:
        const = ctx.enter_context(tc.tile_pool(name="const", bufs=1))
        xt_pool = ctx.enter_context(tc.tile_pool(name="xt", bufs=16))
        qkvt_pool = ctx.enter_context(tc.tile_pool(name="qkvt", bufs=2))
        vn_pool = ctx.enter_context(tc.tile_pool(name="vn", bufs=2))
        pt_pool = ctx.enter_context(tc.tile_pool(name="pt", bufs=4))
        ofin_pool = ctx.enter_context(tc.tile_pool(name="ofin", bufs=2))
        sm_pool = ctx.enter_context(tc.tile_pool(name="sm", bufs=4))
        yo_pool = ctx.enter_context(tc.tile_pool(name="yo", bufs=6))
        ps512 = ctx.enter_context(tc.tile_pool(name="ps512", bufs=2, space="PSUM"))
        pst = ctx.enter_context(tc.tile_pool(name="pst", bufs=2, space="PSUM"))
        pso = ctx.enter_context(tc.tile_pool(name="pso", bufs=2, space="PSUM"))

        # ---- constants ----
        w_sb = const.tile([P, DCH * 3 * P], f32r, tag="w")
        for d in range(DCH):
            nc.sync.dma_start(
                w_sb[:, d * 384 : (d + 1) * 384], wqkv_d[d * P : (d + 1) * P, :]
            )
        wp_sb = const.tile([P, D], f32r, tag="wp")
        nc.sync.dma_start(wp_sb[:], wp_d[:])
        bq_sb = const.tile([P, 3], f32, tag="bq")
        nc.sync.dma_start(bq_sb[:], bq_d[:])
        ident = const.tile([P, P], f32, tag="ident")
        make_identity(nc, ident)
        # ones column for the V-augmentation (softmax denominator row)
        vcol_f = const.tile([P, 1], f32, tag="vcol_f")
        nc.vector.memset(vcol_f[:], 1.0)
        # ones row — broadcasts a [1, QB] reciprocal row across 64 partitions
        ones_f = const.tile([1, DH], f32, tag="ones_f")
        nc.vector.memset(ones_f[:], 1.0)
        ones_sb = const.tile([1, DH], f32r, tag="ones")
        nc.vector.tensor_copy(ones_sb[:], ones_f[:])

        nA = ntok // QB  # token groups (stage A) == query blocks (stage B)

        def stage_a_tok(bt, tb):
            """QKV^T for token block tb of batch bt."""
            t0 = bt * ntok
            qt, kt, vt = bt_tiles[bt]
            dest = {0: qt, 1: kt, 2: vt}
            xts = []
            for d in range(DCH):
                t = xt_pool.tile([P, QB], f32r, tag="xt", name=f"xt{d}")
                nc.sync.dma_start(
                    t[:], xt_d[d * P : (d + 1) * P, t0 + tb * QB : t0 + (tb + 1) * QB]
                )
                xts.append(t)
            for cb in range(3):
                ps = ps512.tile([P, QB], f32, tag="ps", name="psA")
                for d in range(DCH):
                    nc.tensor.matmul(
                        ps[:],
                        w_sb[:, d * 384 + cb * P : d * 384 + (cb + 1) * P],
                        xts[d][:],
                        start=(d == 0),
                        stop=(d == DCH - 1),
                    )
                # psum -> sbuf, adding the (per-output-column) qkv bias
                nc.vector.tensor_scalar_add(
                    dest[cb][:, tb * QB : (tb + 1) * QB], ps[:], bq_sb[:, cb : cb + 1]
                )

        def build_vn_group(bt, g, ngroups):
            """PE-transpose V^T -> V natural (+ones col), for key blocks of group g."""
            _, _, vt = bt_tiles[bt]
            vn = vn_tiles[bt]
            for kb in range(g * nkb // ngroups, (g + 1) * nkb // ngroups):
                trp = ps512.tile([P, P], f32, tag="ps", name="trp")
                nc.tensor.transpose(trp[:], vt[:, kb * P : (kb + 1) * P], ident[:])
                for h in range(HPC):
                    nc.vector.tensor_copy(
                        vn[:, (h * nkb + kb) * 65 : (h * nkb + kb) * 65 + 64],
                        trp[:, h * 64 : (h + 1) * 64],
                    )
                for h in range(HPC):
                    idx = (h * nkb + kb + 1) * 65 - 1
                    nc.vector.tensor_copy(vn[:, idx : idx + 1], vcol_f[:])

        def stage_b_qblock(bt, qb):
            """Attention for query block qb of batch bt, both heads."""
            qt, kt, _ = bt_tiles[bt]
            vn = vn_tiles[bt]
            ofin = ofin_tiles[bt]
            q0, q1 = qb * QB, (qb + 1) * QB
            oacc = [
                pso.tile([65, QB], f32, tag="oacc", name=f"oacc{_h}")
                for _h in range(HPC)
            ]
            rc = [
                sm_pool.tile([1, QB], f32r, tag=f"rc{_h}", name=f"rc{_h}")
                for _h in range(HPC)
            ]
            def emit_o(ptile, kb):
                for h in range(HPC):
                    nc.tensor.matmul(
                        oacc[h][:],
                        vn[:, (h * nkb + kb) * 65 : (h * nkb + kb + 1) * 65],
                        ptile[:, h * QB : (h + 1) * QB],
                        start=(kb == 0),
                        stop=(kb == nkb - 1),
                        skip_group_check=True,
                    )

            # O matmuls are delayed one kb so the PE stream runs S(kb+1)
            # while ACT computes exp(kb) — O(kb) never stalls on its exp.
            pend = None
            for kb in range(nkb):
                st = pst.tile([P, HPC * QB], f32, tag="st", name="st")
                for h in range(HPC):
                    nc.tensor.matmul(
                        st[:, h * QB : (h + 1) * QB],
                        kt[h * 64 : (h + 1) * 64, kb * P : (kb + 1) * P],
                        qt[h * 64 : (h + 1) * 64, q0:q1],
                        start=True,
                        stop=True,
                        tile_position=(h * 64, 0),
                    )
                if pend is not None:
                    emit_o(*pend)
                ptile = pt_pool.tile([P, HPC * QB], f32r, tag="pt", name="pt")
                nc.scalar.activation(ptile[:], st[:], Exp)
                pend = (ptile, kb)
            emit_o(*pend)
            bc_sb = sm_pool.tile([P, QB], f32, tag="bc", name="bc_sb")
            for h in range(HPC):
                with nc.allow_low_precision(reason="f32r feed to bcast matmul"):
                    nc.vector.reciprocal(rc[h][:], oacc[h][64:65, :])
                bc_ps = ps512.tile([DH, QB], f32, tag="ps", name=f"bc_ps{h}")
                nc.tensor.matmul(bc_ps[:], ones_sb[:], rc[h][:], start=True, stop=True)
                nc.vector.tensor_copy(bc_sb[h * 64 : (h + 1) * 64, :], bc_ps[:])
            for h in range(HPC):
                nc.vector.tensor_mul(
                    ofin[h * 64 : (h + 1) * 64, q0:q1],
                    oacc[h][0:64, :],
                    bc_sb[h * 64 : (h + 1) * 64, :],
                )

        def stage_c_group(bt, g, ngroups):
            """Partial projection for token blocks of group g (needs ofin of the
            matching qblock only)."""
            t0 = bt * ntok
            ofin = ofin_tiles[bt]
            for tb in range(g * ntb // ngroups, (g + 1) * ntb // ngroups):
                for eb in range(neb):
                    yp = ps512.tile([P, QB], f32, tag="ps", name="yp")
                    nc.tensor.matmul(
                        yp[:],
                        ofin[:, tb * P : (tb + 1) * P],
                        wp_sb[:, eb * QB : (eb + 1) * QB],
                        start=True,
                        stop=True,
                    )
                    yo = yo_pool.tile([P, QB], f32, tag="yo", name="yo")
                    nc.vector.tensor_copy(yo[:], yp[:])
                    nc.sync.dma_start(
                        y_d[t0 + tb * P : t0 + (tb + 1) * P, eb * QB : (eb + 1) * QB],
                        yo[:],
                    )

        # ---- software-pipelined emission: stage A of batch b+1 interleaves
        # with stage B/C of batch b so every engine's (in-order) stream
        # alternates phases and DMA/PE/ACT overlap across batches ----
        bt_tiles = {}
        vn_tiles = {}
        ofin_tiles = {}
        for step in range(nbatch + 1):
            if step < nbatch:
                qt = qkvt_pool.tile([P, ntok], f32r, tag="qt", name="qt")
                kt = qkvt_pool.tile([P, ntok], f32r, tag="kt", name="kt")
                vt = qkvt_pool.tile([P, ntok], f32, tag="vt", name="vt")
                bt_tiles[step] = (qt, kt, vt)
                vn_tiles[step] = vn_pool.tile(
                    [P, HPC * nkb * 65], f32r, tag="vn", name="vn"
                )
                ofin_tiles[step] = ofin_pool.tile(
                    [P, ntok], f32r, tag="ofin", name="ofin"
                )
            for i in range(nA):
                if step < nbatch:
                    stage_a_tok(step, i)
                    build_vn_group(step, i, nA)
                if step >= 1:
                    stage_b_qblock(step - 1, i)
                    stage_c_group(step - 1, i, nA)
            # free batch tiles we no longer need
            if step >= 1:
                for dct in (bt_tiles, vn_tiles, ofin_tiles):
                    dct.pop(step - 1, None)

    nc.compile()
    return nc


def get_compiled(nbatch=B, ntok=N):
    key = (nbatch, ntok)
    if key not in _cache:
        _cache[key] = _build(nbatch, ntok)
    return _cache[key]


def make_core_inputs(x, w_qkv, b_qkv, w_proj):
    """Host-side sharding: returns (in_maps list for 8 cores)."""
    B_, N_, D_ = x.shape
    xt = np.ascontiguousarray(x.reshape(B_ * N_, D_).T).astype(np.float32)
    in_maps = []
    for c in range(NCORES):
        heads = [HPC * c + i for i in range(HPC)]

        def wcols(s, scale=1.0):
            return np.concatenate(
                [w_qkv[:, s * D + h * DH : s * D + (h + 1) * DH] for h in heads], axis=1
            ) * scale

        def bcol(s, scale=1.0):
            return np.concatenate(
                [b_qkv[s * D + h * DH : s * D + (h + 1) * DH] for h in heads]
            ) * scale

        scale = float(DH) ** -0.5
        wqkv_c = np.ascontiguousarray(
            np.concatenate([wcols(0, scale), wcols(1), wcols(2)], axis=1)
        ).astype(np.float32)
        bq_c = np.stack([bcol(0, scale), bcol(1), bcol(2)], axis=1).astype(np.float32)
        bq_c = np.ascontiguousarray(bq_c)
        wp_c = np.ascontiguousarray(
            np.concatenate([w_proj[h * DH : (h + 1) * DH, :] for h in heads], axis=0)
        ).astype(np.float32)
        in_maps.append({"xt": xt, "wqkv": wqkv_c, "bq": bq_c, "wp": wp_c})
    return in_maps


def kernel(x, w_qkv, b_qkv, w_proj, b_proj):
    x = np.asarray(x, dtype=np.float32)
    w_qkv = np.asarray(w_qkv, dtype=np.float32)
    b_qkv = np.asarray(b_qkv, dtype=np.float32)
    w_proj = np.asarray(w_proj, dtype=np.float32)
    b_proj = np.asarray(b_proj, dtype=np.float32)
    B_, N_, D_ = x.shape

    from concourse.bass_utils import run_bass_kernel_spmd

    nc = get_compiled(B_, N_)
    in_maps = make_core_inputs(x, w_qkv, b_qkv, w_proj)
    res = run_bass_kernel_spmd(nc, in_maps, core_ids=list(range(NCORES)))
    y = res.results[0]["y"].astype(np.float64)
    for r in res.results[1:]:
        y = y + r["y"].astype(np.float64)
    y = y + b_proj[None, :].astype(np.float64)
    return y.reshape(B_, N_, D_).astype(np.float32)


# revision 32
# speedup vs baseline: 10.4510x; 1.0227x over previous
"""Multi-head attention (B=4, N=2048, D=1024, H=16) on 8 Trainium2 NeuronCores.

Sharding: tensor-parallel over heads — 2 heads per core. Each core computes
QKV^T for its heads from the (host-pre-transposed) full X^T, runs attention,
and produces a partial projection output (its 128 rows of w_proj). The host
sums the 8 partial outputs.

Layout trick: everything stays "transposed" (feature dim on partitions):
  QKV^T [128=(h0 dims|h1 dims), tok] = W_chunk.T @ XT_chunk     (fp32r, N=512)
  S^T   [keys, q]  = (K^T_h chunk).T @ Q^T_h                    (64-contraction,
                      two heads packed in array row halves via tile_position)
  P^T   = exp(S^T)          (ScalarE, no max subtraction: logits ~ N(0,1))
  O^T   [65, q]    = V_nat_aug.T @ P^T   (V natural layout via PE transpose,
                      65th row = ones column -> softmax denominator for free)
  Y     [tok, 512] = Ofin_chunk.T @ Wp   (128-contraction per core)
"""

import sys
from contextlib import ExitStack

import numpy as np

for _p in ("/opt/trn_rl_repo", "/opt/pypackages"):
    if _p not in sys.path:
        sys.path.insert(0, _p)

B, N, D = 4, 2048, 1024
H, DH = 16, 64
NCORES = 8
HPC = H // NCORES  # heads per core = 2
P = 128
QB = 512  # moving free dim (query block / token block)

_cache = {}


def _build(nbatch, ntok):
    """Build + compile the per-core bass program. Same program on all cores;
    per-core weights arrive as data."""
    import concourse.bacc as bacc
    import concourse.mybir as mybir
    import concourse.tile as tile
    from concourse.masks import make_identity

    f32 = mybir.dt.float32
    f32r = mybir.dt.float32r
    Exp = mybir.ActivationFunctionType.Exp

    DCH = D // P          # 8 contraction chunks for the projections
    nqb = ntok // QB      # query blocks per batch
    nkb = ntok // P       # key blocks per batch
    ntb = ntok // P       # stage-C token blocks per batch
    neb = D // QB         # output col blocks (2)

    nc = bacc.Bacc("TRN2", target_bir_lowering=False, debug=False)

    xt_d = nc.dram_tensor("xt", [D, nbatch * ntok], f32r, kind="ExternalInput")
    wqkv_d = nc.dram_tensor("wqkv", [D, 3 * P], f32r, kind="ExternalInput")
    bq_d = nc.dram_tensor("bq", [P, 3], f32, kind="ExternalInput")
    wp_d = nc.dram_tensor("wp", [P, D], f32r, kind="ExternalInput")
    y_d = nc.dram_tensor("y", [nbatch * ntok, D], f32, kind="ExternalOutput")

    with tile.TileContext(nc) as tc, ExitStack() as ctx:
        const = ctx.enter_context(tc.tile_pool(name="const", bufs=1))
        xt_pool = ctx.enter_context(tc.tile_pool(name="xt", bufs=16))
        qkvt_pool = ctx.enter_context(tc.tile_pool(name="qkvt", bufs=2))
        vn_pool = ctx.enter_context(tc.tile_pool(name="vn", bufs=2))
        pt_pool = ctx.enter_context(tc.tile_pool(name="pt", bufs=4))
        ofin_pool = ctx.enter_context(tc.tile_pool(name="ofin", bufs=2))
        sm_pool = ctx.enter_context(tc.tile_pool(name="sm", bufs=4))
        yo_pool = ctx.enter_context(tc.tile_pool(name="yo", bufs=6))
        ps512 = ctx.enter_context(tc.tile_pool(name="ps512", bufs=2, space="PSUM"))
        pst = ctx.enter_context(tc.tile_pool(name="pst", bufs=2, space="PSUM"))
        pso = ctx.enter_context(tc.tile_pool(name="pso", bufs=2, space="PSUM"))

        # ---- constants ----
        w_sb = const.tile([P, DCH * 3 * P], f32r, tag="w")
        for d in range(DCH):
            nc.sync.dma_start(
                w_sb[:, d * 384 : (d + 1) * 384], wqkv_d[d * P : (d + 1) * P, :]
            )
        wp_sb = const.tile([P, D], f32r, tag="wp")
        nc.sync.dma_start(wp_sb[:], wp_d[:])
        bq_sb = const.tile([P, 3], f32, tag="bq")
        nc.sync.dma_start(bq_sb[:], bq_d[:])
        ident = const.tile([P, P], f32, tag="ident")
        make_identity(nc, ident)
        # ones column for the V-augmentation (softmax denominator row)
        vcol_f = const.tile([P, 1], f32, tag="vcol_f")
        nc.vector.memset(vcol_f[:], 1.0)
        # ones row — broadcasts a [1, QB] reciprocal row across 64 partitions
        ones_f = const.tile([1, DH], f32, tag="ones_f")
        nc.vector.memset(ones_f[:], 1.0)
        ones_sb = const.tile([1, DH], f32r, tag="ones")
        nc.vector.tensor_copy(ones_sb[:], ones_f[:])

        nA = ntok // QB  # token groups (stage A) == query blocks (stage B)

        def stage_a_tok(bt, tb):
            """QKV^T for token block tb of batch bt."""
            t0 = bt * ntok
            qt, kt, vt = bt_tiles[bt]
            dest = {0: qt, 1: kt, 2: vt}
            xts = []
            for d in range(DCH):
                t = xt_pool.tile([P, QB], f32r, tag="xt", name=f"xt{d}")
                nc.sync.dma_start(
                    t[:], xt_d[d * P : (d + 1) * P, t0 + tb * QB : t0 + (tb + 1) * QB]
                )
                xts.append(t)
            for cb in range(3):
                ps = ps512.tile([P, QB], f32, tag="ps", name="psA")
                for d in range(DCH):
                    nc.tensor.matmul(
                        ps[:],
                        w_sb[:, d * 384 + cb * P : d * 384 + (cb + 1) * P],
                        xts[d][:],
                        start=(d == 0),
                        stop=(d == DCH - 1),
                    )
                # psum -> sbuf, adding the (per-output-column) qkv bias
                nc.vector.tensor_scalar_add(
                    dest[cb][:, tb * QB : (tb + 1) * QB], ps[:], bq_sb[:, cb : cb + 1]
                )

        def build_vn_group(bt, g, ngroups):
            """PE-transpose V^T -> V natural (+ones col), for key blocks of group g."""
            _, _, vt = bt_tiles[bt]
            vn = vn_tiles[bt]
            for kb in range(g * nkb // ngroups, (g + 1) * nkb // ngroups):
                trp = ps512.tile([P, P], f32, tag="ps", name="trp")
                nc.tensor.transpose(trp[:], vt[:, kb * P : (kb + 1) * P], ident[:])
                for h in range(HPC):
                    nc.vector.tensor_copy(
                        vn[:, (h * nkb + kb) * 65 : (h * nkb + kb) * 65 + 64],
                        trp[:, h * 64 : (h + 1) * 64],
                    )
                for h in range(HPC):
                    idx = (h * nkb + kb + 1) * 65 - 1
                    nc.vector.tensor_copy(vn[:, idx : idx + 1], vcol_f[:])

        def stage_b_qblock(bt, qb):
            """Attention for query block qb of batch bt, both heads."""
            qt, kt, _ = bt_tiles[bt]
            vn = vn_tiles[bt]
            ofin = ofin_tiles[bt]
            q0, q1 = qb * QB, (qb + 1) * QB
            oacc = [
                pso.tile([65, QB], f32, tag="oacc", name=f"oacc{_h}")
                for _h in range(HPC)
            ]
            rc = [
                sm_pool.tile([1, QB], f32r, tag=f"rc{_h}", name=f"rc{_h}")
                for _h in range(HPC)
            ]
            def emit_o(ptile, kb):
                for h in range(HPC):
                    nc.tensor.matmul(
                        oacc[h][:],
                        vn[:, (h * nkb + kb) * 65 : (h * nkb + kb + 1) * 65],
                        ptile[:, h * QB : (h + 1) * QB],
                        start=(kb == 0),
                        stop=(kb == nkb - 1),
                        skip_group_check=True,
                    )

            # O matmuls are delayed one kb so the PE stream runs S(kb+1)
            # while ACT computes exp(kb) — O(kb) never stalls on its exp.
            pend = None
            for kb in range(nkb):
                st = pst.tile([P, HPC * QB], f32, tag="st", name="st")
                for h in range(HPC):
                    nc.tensor.matmul(
                        st[:, h * QB : (h + 1) * QB],
                        kt[h * 64 : (h + 1) * 64, kb * P : (kb + 1) * P],
                        qt[h * 64 : (h + 1) * 64, q0:q1],
                        start=True,
                        stop=True,
                        tile_position=(h * 64, 0),
                    )
                if pend is not None:
                    emit_o(*pend)
                ptile = pt_pool.tile([P, HPC * QB], f32r, tag="pt", name="pt")
                nc.scalar.activation(ptile[:], st[:], Exp)
                pend = (ptile, kb)
            emit_o(*pend)
            # early-evict oacc PSUM -> SBUF so the pso slots free for the next
            # qblock's accumulation before the normalization chain completes
            osb = [
                sm_pool.tile([65, QB], f32, tag=f"osb{_h}", name=f"osb{_h}")
                for _h in range(HPC)
            ]
            for h in range(HPC):
                nc.vector.tensor_copy(osb[h][:], oacc[h][:])
            for h in range(HPC):
                with nc.allow_low_precision(reason="f32r feed to bcast matmul"):
                    nc.vector.reciprocal(rc[h][:], osb[h][64:65, :])
                bc_ps = ps512.tile([DH, QB], f32, tag="ps", name=f"bc_ps{h}")
                nc.tensor.matmul(bc_ps[:], ones_sb[:], rc[h][:], start=True, stop=True)
                bch = sm_pool.tile([DH, QB], f32, tag=f"bch{h}", name=f"bch{h}")
                nc.vector.tensor_copy(bch[:], bc_ps[:])
                nc.vector.tensor_mul(
                    ofin[h * 64 : (h + 1) * 64, q0:q1],
                    osb[h][0:64, :],
                    bch[:],
                )

        def stage_c_group(bt, g, ngroups):
            """Partial projection for token blocks of group g (needs ofin of the
            matching qblock only)."""
            t0 = bt * ntok
            ofin = ofin_tiles[bt]
            for tb in range(g * ntb // ngroups, (g + 1) * ntb // ngroups):
                for eb in range(neb):
                    yp = ps512.tile([P, QB], f32, tag="ps", name="yp")
                    nc.tensor.matmul(
                        yp[:],
                        ofin[:, tb * P : (tb + 1) * P],
                        wp_sb[:, eb * QB : (eb + 1) * QB],
                        start=True,
                        stop=True,
                    )
                    yo = yo_pool.tile([P, QB], f32, tag="yo", name="yo")
                    nc.vector.tensor_copy(yo[:], yp[:])
                    nc.sync.dma_start(
                        y_d[t0 + tb * P : t0 + (tb + 1) * P, eb * QB : (eb + 1) * QB],
                        yo[:],
                    )

        # ---- software-pipelined emission: stage A of batch b+1 interleaves
        # with stage B/C of batch b so every engine's (in-order) stream
        # alternates phases and DMA/PE/ACT overlap across batches ----
        bt_tiles = {}
        vn_tiles = {}
        ofin_tiles = {}
        for step in range(nbatch + 1):
            if step < nbatch:
                qt = qkvt_pool.tile([P, ntok], f32r, tag="qt", name="qt")
                kt = qkvt_pool.tile([P, ntok], f32r, tag="kt", name="kt")
                vt = qkvt_pool.tile([P, ntok], f32, tag="vt", name="vt")
                bt_tiles[step] = (qt, kt, vt)
                vn_tiles[step] = vn_pool.tile(
                    [P, HPC * nkb * 65], f32r, tag="vn", name="vn"
                )
                ofin_tiles[step] = ofin_pool.tile(
                    [P, ntok], f32r, tag="ofin", name="ofin"
                )
            for i in range(nA):
                if step < nbatch:
                    stage_a_tok(step, i)
                    build_vn_group(step, i, nA)
                if step >= 1:
                    stage_b_qblock(step - 1, i)
                    stage_c_group(step - 1, i, nA)
            # free batch tiles we no longer need
            if step >= 1:
                for dct in (bt_tiles, vn_tiles, ofin_tiles):
                    dct.pop(step - 1, None)

    nc.compile()
    return nc


def get_compiled(nbatch=B, ntok=N):
    key = (nbatch, ntok)
    if key not in _cache:
        _cache[key] = _build(nbatch, ntok)
    return _cache[key]


def make_core_inputs(x, w_qkv, b_qkv, w_proj):
    """Host-side sharding: returns (in_maps list for 8 cores)."""
    B_, N_, D_ = x.shape
    xt = np.ascontiguousarray(x.reshape(B_ * N_, D_).T).astype(np.float32)
    in_maps = []
    for c in range(NCORES):
        heads = [HPC * c + i for i in range(HPC)]

        def wcols(s, scale=1.0):
            return np.concatenate(
                [w_qkv[:, s * D + h * DH : s * D + (h + 1) * DH] for h in heads], axis=1
            ) * scale

        def bcol(s, scale=1.0):
            return np.concatenate(
                [b_qkv[s * D + h * DH : s * D + (h + 1) * DH] for h in heads]
            ) * scale

        scale = float(DH) ** -0.5
        wqkv_c = np.ascontiguousarray(
            np.concatenate([wcols(0, scale), wcols(1), wcols(2)], axis=1)
        ).astype(np.float32)
        bq_c = np.stack([bcol(0, scale), bcol(1), bcol(2)], axis=1).astype(np.float32)
        bq_c = np.ascontiguousarray(bq_c)
        wp_c = np.ascontiguousarray(
            np.concatenate([w_proj[h * DH : (h + 1) * DH, :] for h in heads], axis=0)
        ).astype(np.float32)
        in_maps.append({"xt": xt, "wqkv": wqkv_c, "bq": bq_c, "wp": wp_c})
    return in_maps


def kernel(x, w_qkv, b_qkv, w_proj, b_proj):
    x = np.asarray(x, dtype=np.float32)
    w_qkv = np.asarray(w_qkv, dtype=np.float32)
    b_qkv = np.asarray(b_qkv, dtype=np.float32)
    w_proj = np.asarray(w_proj, dtype=np.float32)
    b_proj = np.asarray(b_proj, dtype=np.float32)
    B_, N_, D_ = x.shape

    from concourse.bass_utils import run_bass_kernel_spmd

    nc = get_compiled(B_, N_)
    in_maps = make_core_inputs(x, w_qkv, b_qkv, w_proj)
    res = run_bass_kernel_spmd(nc, in_maps, core_ids=list(range(NCORES)))
    y = res.results[0]["y"].astype(np.float64)
    for r in res.results[1:]:
        y = y + r["y"].astype(np.float64)
    y = y + b_proj[None, :].astype(np.float64)
    return y.reshape(B_, N_, D_).astype(np.float32)


# revision 35
# speedup vs baseline: 10.5147x; 1.0061x over previous
"""Multi-head attention (B=4, N=2048, D=1024, H=16) on 8 Trainium2 NeuronCores.

Sharding: tensor-parallel over heads — 2 heads per core. Each core computes
QKV^T for its heads from the (host-pre-transposed) full X^T, runs attention,
and produces a partial projection output (its 128 rows of w_proj). The host
sums the 8 partial outputs.

Layout trick: everything stays "transposed" (feature dim on partitions):
  QKV^T [128=(h0 dims|h1 dims), tok] = W_chunk.T @ XT_chunk     (fp32r, N=512)
  S^T   [keys, q]  = (K^T_h chunk).T @ Q^T_h                    (64-contraction,
                      two heads packed in array row halves via tile_position)
  P^T   = exp(S^T)          (ScalarE, no max subtraction: logits ~ N(0,1))
  O^T   [65, q]    = V_nat_aug.T @ P^T   (V natural layout via PE transpose,
                      65th row = ones column -> softmax denominator for free)
  Y     [tok, 512] = Ofin_chunk.T @ Wp   (128-contraction per core)
"""

import sys
from contextlib import ExitStack

import numpy as np

for _p in ("/opt/trn_rl_repo", "/opt/pypackages"):
    if _p not in sys.path:
        sys.path.insert(0, _p)

B, N, D = 4, 2048, 1024
H, DH = 16, 64
NCORES = 8
HPC = H // NCORES  # heads per core = 2
P = 128
QB = 512  # moving free dim (query block / token block)

_cache = {}


def _build(nbatch, ntok):
    """Build + compile the per-core bass program. Same program on all cores;
    per-core weights arrive as data."""
    import concourse.bacc as bacc
    import concourse.mybir as mybir
    import concourse.tile as tile
    from concourse.masks import make_identity

    f32 = mybir.dt.float32
    f32r = mybir.dt.float32r
    Exp = mybir.ActivationFunctionType.Exp

    DCH = D // P          # 8 contraction chunks for the projections
    nqb = ntok // QB      # query blocks per batch
    nkb = ntok // P       # key blocks per batch
    ntb = ntok // P       # stage-C token blocks per batch
    neb = D // QB         # output col blocks (2)

    nc = bacc.Bacc("TRN2", target_bir_lowering=False, debug=False)

    xt_d = nc.dram_tensor("xt", [D, nbatch * ntok], f32r, kind="ExternalInput")
    wqkv_d = nc.dram_tensor("wqkv", [D, 3 * P], f32r, kind="ExternalInput")
    bq_d = nc.dram_tensor("bq", [P, 3], f32, kind="ExternalInput")
    wp_d = nc.dram_tensor("wp", [P, D], f32r, kind="ExternalInput")
    y_d = nc.dram_tensor("y", [nbatch * ntok, D], f32, kind="ExternalOutput")

    with tile.TileContext(nc) as tc, ExitStack() as ctx:
        const = ctx.enter_context(tc.tile_pool(name="const", bufs=1))
        xt_pool = ctx.enter_context(tc.tile_pool(name="xt", bufs=16))
        qkvt_pool = ctx.enter_context(tc.tile_pool(name="qkvt", bufs=2))
        vn_pool = ctx.enter_context(tc.tile_pool(name="vn", bufs=2))
        pt_pool = ctx.enter_context(tc.tile_pool(name="pt", bufs=4))
        ofin_pool = ctx.enter_context(tc.tile_pool(name="ofin", bufs=2))
        sm_pool = ctx.enter_context(tc.tile_pool(name="sm", bufs=4))
        yo_pool = ctx.enter_context(tc.tile_pool(name="yo", bufs=6))
        ps512 = ctx.enter_context(tc.tile_pool(name="ps512", bufs=2, space="PSUM"))
        pst = ctx.enter_context(tc.tile_pool(name="pst", bufs=2, space="PSUM"))
        pso = ctx.enter_context(tc.tile_pool(name="pso", bufs=2, space="PSUM"))

        # ---- constants ----
        w_sb = const.tile([P, DCH * 3 * P], f32r, tag="w")
        for d in range(DCH):
            nc.sync.dma_start(
                w_sb[:, d * 384 : (d + 1) * 384], wqkv_d[d * P : (d + 1) * P, :]
            )
        wp_sb = const.tile([P, D], f32r, tag="wp")
        nc.sync.dma_start(wp_sb[:], wp_d[:])
        bq_sb = const.tile([P, 3], f32, tag="bq")
        nc.sync.dma_start(bq_sb[:], bq_d[:])
        ident = const.tile([P, P], f32, tag="ident")
        make_identity(nc, ident)
        # ones column for the V-augmentation (softmax denominator row)
        vcol_f = const.tile([P, 1], f32, tag="vcol_f")
        nc.vector.memset(vcol_f[:], 1.0)
        # ones row — broadcasts a [1, QB] reciprocal row across 64 partitions
        ones_f = const.tile([1, DH], f32, tag="ones_f")
        nc.vector.memset(ones_f[:], 1.0)
        ones_sb = const.tile([1, DH], f32r, tag="ones")
        nc.vector.tensor_copy(ones_sb[:], ones_f[:])

        nA = ntok // QB  # token groups (stage A) == query blocks (stage B)

        def stage_a_tok(bt, tb):
            """QKV^T for token block tb of batch bt."""
            t0 = bt * ntok
            qt, kt, vt = bt_tiles[bt]
            dest = {0: qt, 1: kt, 2: vt}
            xts = []
            for d in range(DCH):
                t = xt_pool.tile([P, QB], f32r, tag="xt", name=f"xt{d}")
                nc.sync.dma_start(
                    t[:], xt_d[d * P : (d + 1) * P, t0 + tb * QB : t0 + (tb + 1) * QB]
                )
                xts.append(t)
            for cb in range(3):
                ps = ps512.tile([P, QB], f32, tag="ps", name="psA")
                for d in range(DCH):
                    nc.tensor.matmul(
                        ps[:],
                        w_sb[:, d * 384 + cb * P : d * 384 + (cb + 1) * P],
                        xts[d][:],
                        start=(d == 0),
                        stop=(d == DCH - 1),
                    )
                # psum -> sbuf, adding the (per-output-column) qkv bias
                nc.vector.tensor_scalar_add(
                    dest[cb][:, tb * QB : (tb + 1) * QB], ps[:], bq_sb[:, cb : cb + 1]
                )

        def build_vn_group(bt, g, ngroups):
            """PE-transpose V^T -> V natural (+ones col), for key blocks of group g."""
            _, _, vt = bt_tiles[bt]
            vn = vn_tiles[bt]
            for kb in range(g * nkb // ngroups, (g + 1) * nkb // ngroups):
                trp = ps512.tile([P, P], f32, tag="ps", name="trp")
                nc.tensor.transpose(trp[:], vt[:, kb * P : (kb + 1) * P], ident[:])
                for h in range(HPC):
                    nc.vector.tensor_copy(
                        vn[:, (h * nkb + kb) * 65 : (h * nkb + kb) * 65 + 64],
                        trp[:, h * 64 : (h + 1) * 64],
                    )
                for h in range(HPC):
                    idx = (h * nkb + kb + 1) * 65 - 1
                    nc.vector.tensor_copy(vn[:, idx : idx + 1], vcol_f[:])

        def stage_b_qblock(bt, qb):
            """Attention for query block qb of batch bt, both heads."""
            qt, kt, _ = bt_tiles[bt]
            vn = vn_tiles[bt]
            ofin = ofin_tiles[bt]
            q0, q1 = qb * QB, (qb + 1) * QB
            oacc = [
                pso.tile([65, QB], f32, tag="oacc", name=f"oacc{_h}")
                for _h in range(HPC)
            ]
            rc = [
                sm_pool.tile([1, QB], f32r, tag=f"rc{_h}", name=f"rc{_h}")
                for _h in range(HPC)
            ]
            def emit_o(ptile, kb):
                for h in range(HPC):
                    nc.tensor.matmul(
                        oacc[h][:],
                        vn[:, (h * nkb + kb) * 65 : (h * nkb + kb + 1) * 65],
                        ptile[:, h * QB : (h + 1) * QB],
                        start=(kb == 0),
                        stop=(kb == nkb - 1),
                        skip_group_check=True,
                    )

            # O matmuls are delayed one kb so the PE stream runs S(kb+1)
            # while ACT computes exp(kb) — O(kb) never stalls on its exp.
            pend = None
            for kb in range(nkb):
                st = pst.tile([P, HPC * QB], f32, tag="st", name="st")
                for h in range(HPC):
                    nc.tensor.matmul(
                        st[:, h * QB : (h + 1) * QB],
                        kt[h * 64 : (h + 1) * 64, kb * P : (kb + 1) * P],
                        qt[h * 64 : (h + 1) * 64, q0:q1],
                        start=True,
                        stop=True,
                        tile_position=(h * 64, 0),
                    )
                if pend is not None:
                    emit_o(*pend)
                ptile = pt_pool.tile([P, HPC * QB], f32r, tag="pt", name="pt")
                nc.scalar.activation(ptile[:], st[:], Exp)
                pend = (ptile, kb)
            emit_o(*pend)
            # early-evict oacc PSUM -> SBUF so the pso slots free for the next
            # qblock's accumulation before the normalization chain completes
            osb = [
                sm_pool.tile([65, QB], f32, tag=f"osb{_h}", name=f"osb{_h}")
                for _h in range(HPC)
            ]
            for h in range(HPC):
                nc.vector.tensor_copy(osb[h][:], oacc[h][:])
            for h in range(HPC):
                with nc.allow_low_precision(reason="f32r feed to bcast matmul"):
                    nc.vector.reciprocal(rc[h][:], osb[h][64:65, :])
                bc_ps = ps512.tile([DH, QB], f32, tag="ps", name=f"bc_ps{h}")
                nc.tensor.matmul(bc_ps[:], ones_sb[:], rc[h][:], start=True, stop=True)
                # multiply reads the broadcast directly from PSUM (one PSUM +
                # one SBUF operand — no base-partition restriction)
                nc.vector.tensor_mul(
                    ofin[h * 64 : (h + 1) * 64, q0:q1],
                    bc_ps[:],
                    osb[h][0:64, :],
                )

        def stage_c_group(bt, g, ngroups):
            """Partial projection for token blocks of group g (needs ofin of the
            matching qblock only)."""
            t0 = bt * ntok
            ofin = ofin_tiles[bt]
            for tb in range(g * ntb // ngroups, (g + 1) * ntb // ngroups):
                for eb in range(neb):
                    yp = ps512.tile([P, QB], f32, tag="ps", name="yp")
                    nc.tensor.matmul(
                        yp[:],
                        ofin[:, tb * P : (tb + 1) * P],
                        wp_sb[:, eb * QB : (eb + 1) * QB],
                        start=True,
                        stop=True,
                    )
                    yo = yo_pool.tile([P, QB], f32, tag="yo", name="yo")
                    nc.vector.tensor_copy(yo[:], yp[:])
                    nc.sync.dma_start(
                        y_d[t0 + tb * P : t0 + (tb + 1) * P, eb * QB : (eb + 1) * QB],
                        yo[:],
                    )

        # ---- software-pipelined emission: stage A of batch b+1 interleaves
        # with stage B/C of batch b so every engine's (in-order) stream
        # alternates phases and DMA/PE/ACT overlap across batches ----
        bt_tiles = {}
        vn_tiles = {}
        ofin_tiles = {}
        for step in range(nbatch + 1):
            if step < nbatch:
                qt = qkvt_pool.tile([P, ntok], f32r, tag="qt", name="qt")
                kt = qkvt_pool.tile([P, ntok], f32r, tag="kt", name="kt")
                vt = qkvt_pool.tile([P, ntok], f32, tag="vt", name="vt")
                bt_tiles[step] = (qt, kt, vt)
                vn_tiles[step] = vn_pool.tile(
                    [P, HPC * nkb * 65], f32r, tag="vn", name="vn"
                )
                ofin_tiles[step] = ofin_pool.tile(
                    [P, ntok], f32r, tag="ofin", name="ofin"
                )
            for i in range(nA):
                if step < nbatch:
                    stage_a_tok(step, i)
                    build_vn_group(step, i, nA)
                if step >= 1:
                    stage_b_qblock(step - 1, i)
                    stage_c_group(step - 1, i, nA)
            # free batch tiles we no longer need
            if step >= 1:
                for dct in (bt_tiles, vn_tiles, ofin_tiles):
                    dct.pop(step - 1, None)

    nc.compile()
    return nc


def get_compiled(nbatch=B, ntok=N):
    key = (nbatch, ntok)
    if key not in _cache:
        _cache[key] = _build(nbatch, ntok)
    return _cache[key]


def make_core_inputs(x, w_qkv, b_qkv, w_proj):
    """Host-side sharding: returns (in_maps list for 8 cores)."""
    B_, N_, D_ = x.shape
    xt = np.ascontiguousarray(x.reshape(B_ * N_, D_).T).astype(np.float32)
    in_maps = []
    for c in range(NCORES):
        heads = [HPC * c + i for i in range(HPC)]

        def wcols(s, scale=1.0):
            return np.concatenate(
                [w_qkv[:, s * D + h * DH : s * D + (h + 1) * DH] for h in heads], axis=1
            ) * scale

        def bcol(s, scale=1.0):
            return np.concatenate(
                [b_qkv[s * D + h * DH : s * D + (h + 1) * DH] for h in heads]
            ) * scale

        scale = float(DH) ** -0.5
        wqkv_c = np.ascontiguousarray(
            np.concatenate([wcols(0, scale), wcols(1), wcols(2)], axis=1)
        ).astype(np.float32)
        bq_c = np.stack([bcol(0, scale), bcol(1), bcol(2)], axis=1).astype(np.float32)
        bq_c = np.ascontiguousarray(bq_c)
        wp_c = np.ascontiguousarray(
            np.concatenate([w_proj[h * DH : (h + 1) * DH, :] for h in heads], axis=0)
        ).astype(np.float32)
        in_maps.append({"xt": xt, "wqkv": wqkv_c, "bq": bq_c, "wp": wp_c})
    return in_maps


def kernel(x, w_qkv, b_qkv, w_proj, b_proj):
    x = np.asarray(x, dtype=np.float32)
    w_qkv = np.asarray(w_qkv, dtype=np.float32)
    b_qkv = np.asarray(b_qkv, dtype=np.float32)
    w_proj = np.asarray(w_proj, dtype=np.float32)
    b_proj = np.asarray(b_proj, dtype=np.float32)
    B_, N_, D_ = x.shape

    from concourse.bass_utils import run_bass_kernel_spmd

    nc = get_compiled(B_, N_)
    in_maps = make_core_inputs(x, w_qkv, b_qkv, w_proj)
    res = run_bass_kernel_spmd(nc, in_maps, core_ids=list(range(NCORES)))
    y = res.results[0]["y"].astype(np.float64)
    for r in res.results[1:]:
        y = y + r["y"].astype(np.float64)
    y = y + b_proj[None, :].astype(np.float64)
    return y.reshape(B_, N_, D_).astype(np.float32)


# revision 37
# speedup vs baseline: 10.5804x; 1.0062x over previous
"""Multi-head attention (B=4, N=2048, D=1024, H=16) on 8 Trainium2 NeuronCores.

Sharding: tensor-parallel over heads — 2 heads per core. Each core computes
QKV^T for its heads from the (host-pre-transposed) full X^T, runs attention,
and produces a partial projection output (its 128 rows of w_proj). The host
sums the 8 partial outputs.

Layout trick: everything stays "transposed" (feature dim on partitions):
  QKV^T [128=(h0 dims|h1 dims), tok] = W_chunk.T @ XT_chunk     (fp32r, N=512)
  S^T   [keys, q]  = (K^T_h chunk).T @ Q^T_h                    (64-contraction,
                      two heads packed in array row halves via tile_position)
  P^T   = exp(S^T)          (ScalarE, no max subtraction: logits ~ N(0,1))
  O^T   [65, q]    = V_nat_aug.T @ P^T   (V natural layout via PE transpose,
                      65th row = ones column -> softmax denominator for free)
  Y     [tok, 512] = Ofin_chunk.T @ Wp   (128-contraction per core)
"""

import sys
from contextlib import ExitStack

import numpy as np

for _p in ("/opt/trn_rl_repo", "/opt/pypackages"):
    if _p not in sys.path:
        sys.path.insert(0, _p)

B, N, D = 4, 2048, 1024
H, DH = 16, 64
NCORES = 8
HPC = H // NCORES  # heads per core = 2
P = 128
QB = 512  # moving free dim (query block / token block)

_cache = {}


def _build(nbatch, ntok):
    """Build + compile the per-core bass program. Same program on all cores;
    per-core weights arrive as data."""
    import concourse.bacc as bacc
    import concourse.mybir as mybir
    import concourse.tile as tile
    from concourse.masks import make_identity

    f32 = mybir.dt.float32
    f32r = mybir.dt.float32r
    Exp = mybir.ActivationFunctionType.Exp

    DCH = D // P          # 8 contraction chunks for the projections
    nqb = ntok // QB      # query blocks per batch
    nkb = ntok // P       # key blocks per batch
    ntb = ntok // P       # stage-C token blocks per batch
    neb = D // QB         # output col blocks (2)

    nc = bacc.Bacc("TRN2", target_bir_lowering=False, debug=False)

    xt_d = nc.dram_tensor("xt", [D, nbatch * ntok], f32r, kind="ExternalInput")
    wqkv_d = nc.dram_tensor("wqkv", [D, 3 * P], f32r, kind="ExternalInput")
    bq_d = nc.dram_tensor("bq", [P, 3], f32, kind="ExternalInput")
    wp_d = nc.dram_tensor("wp", [P, D], f32r, kind="ExternalInput")
    y_d = nc.dram_tensor("y", [nbatch * ntok, D], f32, kind="ExternalOutput")

    with tile.TileContext(nc) as tc, ExitStack() as ctx:
        const = ctx.enter_context(tc.tile_pool(name="const", bufs=1))
        xt_pool = ctx.enter_context(tc.tile_pool(name="xt", bufs=16))
        qkvt_pool = ctx.enter_context(tc.tile_pool(name="qkvt", bufs=2))
        vn_pool = ctx.enter_context(tc.tile_pool(name="vn", bufs=2))
        pt_pool = ctx.enter_context(tc.tile_pool(name="pt", bufs=4))
        ofin_pool = ctx.enter_context(tc.tile_pool(name="ofin", bufs=2))
        sm_pool = ctx.enter_context(tc.tile_pool(name="sm", bufs=4))
        yo_pool = ctx.enter_context(tc.tile_pool(name="yo", bufs=6))
        ps512 = ctx.enter_context(tc.tile_pool(name="ps512", bufs=2, space="PSUM"))
        pst = ctx.enter_context(tc.tile_pool(name="pst", bufs=2, space="PSUM"))
        pso = ctx.enter_context(tc.tile_pool(name="pso", bufs=2, space="PSUM"))

        # ---- constants ----
        w_sb = const.tile([P, DCH * 3 * P], f32r, tag="w")
        # (w chunk DMAs are interleaved with the first token group's xt DMAs
        # inside stage_a_tok so the first QKV accumulation starts earlier)
        wp_sb = const.tile([P, D], f32r, tag="wp")
        nc.sync.dma_start(wp_sb[:], wp_d[:])
        bq_sb = const.tile([P, 3], f32, tag="bq")
        nc.sync.dma_start(bq_sb[:], bq_d[:])
        ident = const.tile([P, P], f32, tag="ident")
        make_identity(nc, ident)
        # ones column for the V-augmentation (softmax denominator row)
        vcol_f = const.tile([P, 1], f32, tag="vcol_f")
        nc.vector.memset(vcol_f[:], 1.0)
        # ones row — broadcasts a [1, QB] reciprocal row across 64 partitions
        ones_f = const.tile([1, DH], f32, tag="ones_f")
        nc.vector.memset(ones_f[:], 1.0)
        ones_sb = const.tile([1, DH], f32r, tag="ones")
        nc.vector.tensor_copy(ones_sb[:], ones_f[:])

        nA = ntok // QB  # token groups (stage A) == query blocks (stage B)

        def stage_a_tok(bt, tb):
            """QKV^T for token block tb of batch bt."""
            t0 = bt * ntok
            qt, kt, vt = bt_tiles[bt]
            dest = {0: qt, 1: kt, 2: vt}
            xts = []
            for d in range(DCH):
                if bt == 0 and tb == 0:
                    nc.sync.dma_start(
                        w_sb[:, d * 384 : (d + 1) * 384],
                        wqkv_d[d * P : (d + 1) * P, :],
                    )
                t = xt_pool.tile([P, QB], f32r, tag="xt", name=f"xt{d}")
                nc.sync.dma_start(
                    t[:], xt_d[d * P : (d + 1) * P, t0 + tb * QB : t0 + (tb + 1) * QB]
                )
                xts.append(t)
            for cb in range(3):
                ps = ps512.tile([P, QB], f32, tag="ps", name="psA")
                for d in range(DCH):
                    nc.tensor.matmul(
                        ps[:],
                        w_sb[:, d * 384 + cb * P : d * 384 + (cb + 1) * P],
                        xts[d][:],
                        start=(d == 0),
                        stop=(d == DCH - 1),
                    )
                # psum -> sbuf, adding the (per-output-column) qkv bias
                nc.vector.tensor_scalar_add(
                    dest[cb][:, tb * QB : (tb + 1) * QB], ps[:], bq_sb[:, cb : cb + 1]
                )

        def build_vn_group(bt, g, ngroups):
            """PE-transpose V^T -> V natural (+ones col), for key blocks of group g."""
            _, _, vt = bt_tiles[bt]
            vn = vn_tiles[bt]
            for kb in range(g * nkb // ngroups, (g + 1) * nkb // ngroups):
                trp = ps512.tile([P, P], f32, tag="ps", name="trp")
                nc.tensor.transpose(trp[:], vt[:, kb * P : (kb + 1) * P], ident[:])
                for h in range(HPC):
                    nc.vector.tensor_copy(
                        vn[:, (h * nkb + kb) * 65 : (h * nkb + kb) * 65 + 64],
                        trp[:, h * 64 : (h + 1) * 64],
                    )
                for h in range(HPC):
                    idx = (h * nkb + kb + 1) * 65 - 1
                    nc.vector.tensor_copy(vn[:, idx : idx + 1], vcol_f[:])

        def stage_b_qblock(bt, qb):
            """Attention for query block qb of batch bt, both heads."""
            qt, kt, _ = bt_tiles[bt]
            vn = vn_tiles[bt]
            ofin = ofin_tiles[bt]
            q0, q1 = qb * QB, (qb + 1) * QB
            oacc = [
                pso.tile([65, QB], f32, tag="oacc", name=f"oacc{_h}")
                for _h in range(HPC)
            ]
            rc = [
                sm_pool.tile([1, QB], f32r, tag=f"rc{_h}", name=f"rc{_h}")
                for _h in range(HPC)
            ]
            def emit_o(ptile, kb):
                for h in range(HPC):
                    nc.tensor.matmul(
                        oacc[h][:],
                        vn[:, (h * nkb + kb) * 65 : (h * nkb + kb + 1) * 65],
                        ptile[:, h * QB : (h + 1) * QB],
                        start=(kb == 0),
                        stop=(kb == nkb - 1),
                        skip_group_check=True,
                    )

            # O matmuls are delayed one kb so the PE stream runs S(kb+1)
            # while ACT computes exp(kb) — O(kb) never stalls on its exp.
            pend = None
            for kb in range(nkb):
                st = pst.tile([P, HPC * QB], f32, tag="st", name="st")
                for h in range(HPC):
                    nc.tensor.matmul(
                        st[:, h * QB : (h + 1) * QB],
                        kt[h * 64 : (h + 1) * 64, kb * P : (kb + 1) * P],
                        qt[h * 64 : (h + 1) * 64, q0:q1],
                        start=True,
                        stop=True,
                        tile_position=(h * 64, 0),
                    )
                if pend is not None:
                    emit_o(*pend)
                ptile = pt_pool.tile([P, HPC * QB], f32r, tag="pt", name="pt")
                nc.scalar.activation(ptile[:], st[:], Exp)
                pend = (ptile, kb)
            emit_o(*pend)
            # early-evict oacc PSUM -> SBUF so the pso slots free for the next
            # qblock's accumulation before the normalization chain completes
            osb = [
                sm_pool.tile([65, QB], f32, tag=f"osb{_h}", name=f"osb{_h}")
                for _h in range(HPC)
            ]
            for h in range(HPC):
                nc.vector.tensor_copy(osb[h][:], oacc[h][:])
            for h in range(HPC):
                with nc.allow_low_precision(reason="f32r feed to bcast matmul"):
                    nc.vector.reciprocal(rc[h][:], osb[h][64:65, :])
                bc_ps = ps512.tile([DH, QB], f32, tag="ps", name=f"bc_ps{h}")
                nc.tensor.matmul(bc_ps[:], ones_sb[:], rc[h][:], start=True, stop=True)
                # multiply reads the broadcast directly from PSUM (one PSUM +
                # one SBUF operand — no base-partition restriction)
                nc.vector.tensor_mul(
                    ofin[h * 64 : (h + 1) * 64, q0:q1],
                    bc_ps[:],
                    osb[h][0:64, :],
                )

        def stage_c_group(bt, g, ngroups):
            """Partial projection for token blocks of group g (needs ofin of the
            matching qblock only)."""
            t0 = bt * ntok
            ofin = ofin_tiles[bt]
            for tb in range(g * ntb // ngroups, (g + 1) * ntb // ngroups):
                for eb in range(neb):
                    yp = ps512.tile([P, QB], f32, tag="ps", name="yp")
                    nc.tensor.matmul(
                        yp[:],
                        ofin[:, tb * P : (tb + 1) * P],
                        wp_sb[:, eb * QB : (eb + 1) * QB],
                        start=True,
                        stop=True,
                    )
                    yo = yo_pool.tile([P, QB], f32, tag="yo", name="yo")
                    nc.vector.tensor_copy(yo[:], yp[:])
                    nc.sync.dma_start(
                        y_d[t0 + tb * P : t0 + (tb + 1) * P, eb * QB : (eb + 1) * QB],
                        yo[:],
                    )

        # ---- software-pipelined emission: stage A of batch b+1 interleaves
        # with stage B/C of batch b so every engine's (in-order) stream
        # alternates phases and DMA/PE/ACT overlap across batches ----
        bt_tiles = {}
        vn_tiles = {}
        ofin_tiles = {}
        for step in range(nbatch + 1):
            if step < nbatch:
                qt = qkvt_pool.tile([P, ntok], f32r, tag="qt", name="qt")
                kt = qkvt_pool.tile([P, ntok], f32r, tag="kt", name="kt")
                vt = qkvt_pool.tile([P, ntok], f32, tag="vt", name="vt")
                bt_tiles[step] = (qt, kt, vt)
                vn_tiles[step] = vn_pool.tile(
                    [P, HPC * nkb * 65], f32r, tag="vn", name="vn"
                )
                ofin_tiles[step] = ofin_pool.tile(
                    [P, ntok], f32r, tag="ofin", name="ofin"
                )
            for i in range(nA):
                if step < nbatch:
                    stage_a_tok(step, i)
                    build_vn_group(step, i, nA)
                if step >= 1:
                    stage_b_qblock(step - 1, i)
                    stage_c_group(step - 1, i, nA)
            # free batch tiles we no longer need
            if step >= 1:
                for dct in (bt_tiles, vn_tiles, ofin_tiles):
                    dct.pop(step - 1, None)

    nc.compile()
    return nc


def get_compiled(nbatch=B, ntok=N):
    key = (nbatch, ntok)
    if key not in _cache:
        _cache[key] = _build(nbatch, ntok)
    return _cache[key]


def make_core_inputs(x, w_qkv, b_qkv, w_proj):
    """Host-side sharding: returns (in_maps list for 8 cores)."""
    B_, N_, D_ = x.shape
    xt = np.ascontiguousarray(x.reshape(B_ * N_, D_).T).astype(np.float32)
    in_maps = []
    for c in range(NCORES):
        heads = [HPC * c + i for i in range(HPC)]

        def wcols(s, scale=1.0):
            return np.concatenate(
                [w_qkv[:, s * D + h * DH : s * D + (h + 1) * DH] for h in heads], axis=1
            ) * scale

        def bcol(s, scale=1.0):
            return np.concatenate(
                [b_qkv[s * D + h * DH : s * D + (h + 1) * DH] for h in heads]
            ) * scale

        scale = float(DH) ** -0.5
        wqkv_c = np.ascontiguousarray(
            np.concatenate([wcols(0, scale), wcols(1), wcols(2)], axis=1)
        ).astype(np.float32)
        bq_c = np.stack([bcol(0, scale), bcol(1), bcol(2)], axis=1).astype(np.float32)
        bq_c = np.ascontiguousarray(bq_c)
        wp_c = np.ascontiguousarray(
            np.concatenate([w_proj[h * DH : (h + 1) * DH, :] for h in heads], axis=0)
        ).astype(np.float32)
        in_maps.append({"xt": xt, "wqkv": wqkv_c, "bq": bq_c, "wp": wp_c})
    return in_maps


def kernel(x, w_qkv, b_qkv, w_proj, b_proj):
    x = np.asarray(x, dtype=np.float32)
    w_qkv = np.asarray(w_qkv, dtype=np.float32)
    b_qkv = np.asarray(b_qkv, dtype=np.float32)
    w_proj = np.asarray(w_proj, dtype=np.float32)
    b_proj = np.asarray(b_proj, dtype=np.float32)
    B_, N_, D_ = x.shape

    from concourse.bass_utils import run_bass_kernel_spmd

    nc = get_compiled(B_, N_)
    in_maps = make_core_inputs(x, w_qkv, b_qkv, w_proj)
    res = run_bass_kernel_spmd(nc, in_maps, core_ids=list(range(NCORES)))
    y = res.results[0]["y"].astype(np.float64)
    for r in res.results[1:]:
        y = y + r["y"].astype(np.float64)
    y = y + b_proj[None, :].astype(np.float64)
    return y.reshape(B_, N_, D_).astype(np.float32)
